# revision 4
# baseline (speedup 1.0000x reference)
"""Single-head causal attention with RoPE on 8 TRN2 NeuronCores.

Sharding: core c -> batch c//2, parity p = c%2 takes the interleaved
512-row q-blocks {p, p+2, p+4, p+6} of T=4096 (causal load balance).
Each core computes full K/V for its batch (duplicated across the pair),
so no collectives are needed.

Device layout tricks:
- xT passed host-transposed and column-permuted into "slot" order
  [own q-blocks | other blocks] so the SPMD program is identical on all
  cores (q projection always for t-slots 0..15).
- Wq/Wk rows host-permuted evens-first so RoPE becomes rotate-half form
  (free-dim ops only); scores are permutation-invariant.
- Scores computed transposed (S^T[s, q]) so softmax P^T feeds the AV
  matmul directly; row sums via ones-vector matmuls; causal masking via
  exp bias (-1e9) for the data-dependent tail block plus a static
  triangular multiplicative mask for the diagonal block.
"""
import numpy as np

B, T, C, HD = 4, 4096, 2048, 128
P = 128
NB = 8          # 512-row blocks per sequence
BS = 512        # block size
SCALE = float(C) ** -0.5
NEG = -1.0e9


def build():
    import concourse.bass as bass
    import concourse.mybir as mybir
    import bass_rust
    from concourse.tile import TileContext
    from concourse.masks import make_identity

    f32 = mybir.dt.float32
    f32r = mybir.dt.float32r
    EXP = mybir.ActivationFunctionType.Exp

    nc = bass.Bass()
    xt = nc.declare_dram_parameter("xt", [C, T], f32, isOutput=False)
    w = nc.declare_dram_parameter("w", [C, 3 * HD], f32, isOutput=False)
    cos2 = nc.declare_dram_parameter("cos2", [T, P], f32, isOutput=False)
    sin2 = nc.declare_dram_parameter("sin2", [T, P], f32, isOutput=False)
    tailb = nc.declare_dram_parameter("tailb", [P, 1], f32, isOutput=False)
    out = nc.declare_dram_parameter("out", [T // 2, HD], f32, isOutput=True)

    xtr = xt.bitcast(f32r)
    wr = w.bitcast(f32r)

    with TileContext(nc) as tc:
        with (
            tc.tile_pool(name="const", bufs=1) as cp,
            tc.tile_pool(name="xp", bufs=2) as xp,
            tc.tile_pool(name="rot", bufs=2) as rp,
            tc.tile_pool(name="pt", bufs=3) as ptp,
            tc.tile_pool(name="osb", bufs=2) as osb,
            tc.tile_pool(name="pps", bufs=2, space="PSUM") as pps,
            tc.tile_pool(name="tps", bufs=2, space="PSUM") as tps,
            tc.tile_pool(name="sps", bufs=2, space="PSUM") as sps,
            tc.tile_pool(name="o2ps", bufs=1, space="PSUM") as o2ps,
            tc.tile_pool(name="smps", bufs=1, space="PSUM") as smps,
        ):
            # ---- constants / resident tensors ----
            ident = cp.tile([P, P], f32, tag="ident")
            make_identity(nc, ident[:])
            ones = cp.tile([P, 2], f32, tag="ones")
            nc.gpsimd.memset(ones[:], 1.0)
            tri = cp.tile([P, 4 * BS], f32, tag="tri")
            nc.gpsimd.memset(tri[:], 0.0)
            for j in range(4):
                # tri_j[s, q] = 1.0 where s + 128*j <= q else 0.0
                nc.gpsimd.affine_select(
                    out=tri[:, j * BS:(j + 1) * BS],
                    in_=tri[:, j * BS:(j + 1) * BS],
                    compare_op=mybir.AluOpType.is_gt,
                    fill=1.0, base=j * P,
                    pattern=[[-1, BS]], channel_multiplier=1,
                )
            wt = cp.tile([P, 16 * 384], f32r, tag="wt")
            for g in range(4):   # 4 DMAs -> 4 queues
                nc.sync.dma_start(
                    wt[:, g * 4 * 384:(g + 1) * 4 * 384].rearrange(
                        "p (k n) -> p k n", k=4),
                    wr[g * 512:(g + 1) * 512, :].rearrange(
                        "(k p) n -> p k n", p=P))
            cst = cp.tile([P, 32 * P], f32, tag="cst")
            snt = cp.tile([P, 32 * P], f32, tag="snt")
            for g in range(4):
                sl = slice(g * 8 * P, (g + 1) * 8 * P)
                nc.sync.dma_start(
                    cst[:, sl].rearrange("p (k n) -> p k n", k=8),
                    cos2[g * 8 * P:(g + 1) * 8 * P, :].rearrange(
                        "(k p) n -> p k n", p=P))
                nc.sync.dma_start(
                    snt[:, sl].rearrange("p (k n) -> p k n", k=8),
                    sin2[g * 8 * P:(g + 1) * 8 * P, :].rearrange(
                        "(k p) n -> p k n", p=P))
            tb = cp.tile([P, 1], f32, tag="tb")
            nc.sync.dma_start(tb[:], tailb[:])

            qT = cp.tile([P, 16 * P], f32r, tag="qT")   # [d, 2048]
            kT = cp.tile([P, 32 * P], f32r, tag="kT")   # [d, 4096]
            vsb = cp.tile([P, 32 * P], f32r, tag="vsb")  # v[s,d] by s-tile

            # ---- phase 1: joint projection + RoPE + transposes ----
            for tg in range(8):          # t-groups of 512 (slot order)
                xts = []
                for ci in range(16):
                    xtile = xp.tile([P, BS], f32r, tag=f"x{ci}")
                    nc.sync.dma_start(
                        xtile[:], xtr[ci * P:(ci + 1) * P,
                                      tg * BS:(tg + 1) * BS])
                    xts.append(xtile)
                for sub in range(4):
                    t128 = tg * 4 + sub
                    nq = 384 if t128 < 16 else 256   # [k|v|q] layout
                    pp = pps.tile([P, 384], f32, tag="pp")
                    for ci in range(16):
                        nc.tensor.matmul(
                            pp[:, 0:nq],
                            xts[ci][:, sub * P:(sub + 1) * P],
                            wt[:, ci * 384:ci * 384 + nq],
                            start=(ci == 0), stop=(ci == 15))
                    cs = cst[:, t128 * P:(t128 + 1) * P]
                    sn = snt[:, t128 * P:(t128 + 1) * P]
                    H = 64

                    def rope(src_off, dst):
                        s0 = pp[:, src_off:src_off + P]
                        nc.vector.tensor_mul(dst[:], s0, cs)
                        tmp = rp.tile([P, P], f32, tag="ropetmp")
                        nc.vector.tensor_mul(
                            tmp[:, 0:H], pp[:, src_off + H:src_off + P],
                            sn[:, 0:H])
                        nc.vector.tensor_mul(
                            tmp[:, H:P], pp[:, src_off:src_off + H],
                            sn[:, H:P])
                        nc.vector.tensor_add(dst[:], dst[:], tmp[:])

                    rk = rp.tile([P, P], f32, tag="rk")
                    rope(0, rk)
                    nc.scalar.copy(vsb[:, t128 * P:(t128 + 1) * P],
                                   pp[:, P:2 * P])
                    tpk = tps.tile([P, P], f32, tag="tp")
                    nc.tensor.transpose(tpk[:], rk[:], ident[:])
                    nc.scalar.copy(kT[:, t128 * P:(t128 + 1) * P], tpk[:])
                    if t128 < 16:
                        rq = rp.tile([P, P], f32, tag="rq")
                        rope(2 * P, rq)
                        tpq = tps.tile([P, P], f32, tag="tp")
                        nc.tensor.transpose(tpq[:], rq[:], ident[:])
                        nc.scalar.copy(qT[:, t128 * P:(t128 + 1) * P],
                                       tpq[:])

            # ---- phase 2: attention per q-slot ----
            for j in range(4):
                qsl = slice(j * BS, (j + 1) * BS)
                o2 = o2ps.tile([P, BS], f32, tag="o2")
                sm = smps.tile([1, BS], f32, tag="sm")
                slots = ([(s, "full") for s in range(j)]
                         + [(4 + s, "full") for s in range(j)]
                         + [(j, "diag"), (4 + j, "tail")])
                nmm = len(slots) * 4
                mm = 0
                for (si, kind) in slots:
                    for st in range(4):
                        scol = si * BS + st * P
                        Sps = sps.tile([P, BS], f32, tag="S")
                        nc.tensor.matmul(Sps[:], kT[:, scol:scol + P],
                                         qT[:, qsl], start=True, stop=True)
                        Pt = ptp.tile([P, BS], f32r, tag="Pt")
                        bias = tb[:, 0:1] if kind == "tail" else 0.0
                        nc.scalar.activation(Pt[:], Sps[:], EXP,
                                             bias=bias, scale=SCALE)
                        if kind == "diag":
                            nc.vector.tensor_mul(
                                Pt[:], Pt[:], tri[:, st * BS:(st + 1) * BS])
                        nc.tensor.matmul(o2[:], vsb[:, scol:scol + P], Pt[:],
                                         start=(mm == 0), stop=(mm == nmm - 1))
                        nc.tensor.matmul(sm[:], ones[:, 0:1].bitcast(f32r), Pt[:],
                                         start=(mm == 0), stop=(mm == nmm - 1))
                        mm += 1
                # normalize + transpose + store
                smsb = osb.tile([1, BS], f32, tag="smsb")
                nc.scalar.copy(smsb[:], sm[:])
                o2sb = osb.tile([P, BS], f32, tag="o2sb")
                nc.scalar.copy(o2sb[:], o2[:])
                rcp = osb.tile([P, 4], f32, tag="rcp")
                for ch in range(4):
                    rs = tps.tile([P, 1], f32, tag="tp")
                    nc.tensor.transpose(rs[:], smsb[0:1, ch * P:(ch + 1) * P],
                                        ident[0:1, 0:1])
                    nc.vector.reciprocal(rcp[:, ch:ch + 1], rs[:])
                for ch in range(4):
                    ot = tps.tile([P, P], f32, tag="tp")
                    nc.tensor.transpose(ot[:], o2sb[:, ch * P:(ch + 1) * P],
                                        ident[:])
                    osbt = osb.tile([P, P], f32, tag="ofin")
                    nc.vector.tensor_scalar_mul(osbt[:], ot[:],
                                                rcp[:, ch:ch + 1])
                    r0 = j * BS + ch * P
                    nc.sync.dma_start(out[r0:r0 + P, :], osbt[:])

    bass_rust.generate_event_semaphores(nc)
    return nc


_CACHE = {}


def _get_nc():
    if "nc" not in _CACHE:
        _CACHE["nc"] = build()
    return _CACHE["nc"]


def _prep_inputs(x, Wq, Wk, Wv, cos, sin):
    perm = np.concatenate([np.arange(0, HD, 2), np.arange(1, HD, 2)])
    wq = Wq[perm].astype(np.float32)
    wk = Wk[perm].astype(np.float32)
    w = np.concatenate([wk.T, Wv.T.astype(np.float32), wq.T], axis=1)
    w = np.ascontiguousarray(w)  # [C, 384] = [k|v|q]
    cos2 = np.concatenate([cos, cos], axis=1).astype(np.float32)
    sin2 = np.concatenate([-sin, sin], axis=1).astype(np.float32)
    in_maps = []
    orders = []
    for c in range(8):
        b, par = c // 2, c % 2
        order = [par, par + 2, par + 4, par + 6,
                 1 - par, 3 - par, 5 - par, 7 - par]
        orders.append(order)
        xb = np.asarray(x[b], np.float32)          # [T, C]
        xtp = np.empty((C, T), np.float32)
        c2 = np.empty((T, P), np.float32)
        s2 = np.empty((T, P), np.float32)
        for sl, ab in enumerate(order):
            dst = slice(sl * BS, (sl + 1) * BS)
            src = slice(ab * BS, (ab + 1) * BS)
            xtp[:, dst] = xb[src].T
            c2[dst] = cos2[src]
            s2[dst] = sin2[src]
        tailb = np.full((P, 1), NEG if par == 0 else 0.0, np.float32)
        in_maps.append({"xt": np.ascontiguousarray(xtp), "w": w,
                        "cos2": np.ascontiguousarray(c2),
                        "sin2": np.ascontiguousarray(s2), "tailb": tailb})
    return in_maps, orders


def _run(x, Wq, Wk, Wv, cos, sin, trace=False):
    from concourse.bass_utils import run_bass_kernel_spmd
    nc = _get_nc()
    in_maps, orders = _prep_inputs(x, Wq, Wk, Wv, cos, sin)
    res = run_bass_kernel_spmd(nc, in_maps, list(range(8)), trace=trace)
    full = np.empty((B, T, HD), np.float32)
    for c in range(8):
        b, order = c // 2, orders[c]
        oc = res.results[c]["out"]
        for j in range(4):
            ab = order[j]
            full[b, ab * BS:(ab + 1) * BS] = oc[j * BS:(j + 1) * BS]
    return full, res


def kernel(x, Wq, Wk, Wv, cos, sin):
    return _run(x, Wq, Wk, Wv, cos, sin, trace=False)[0]
